# revision 1
# baseline (speedup 1.0000x reference)
"""MoE grouped-GEMM expert FFN (SwiGLU) for Trainium2, 8-core expert parallelism.

Contract: kernel(**inputs) takes FULL unsharded inputs, returns FULL output.

Strategy:
  - Host-side routing: tokens are contiguous per expert; split expert groups
    into chunks, LPT-balance chunks across 8 cores with an identical
    segment-capacity structure on every core (SPMD: one Bass program).
  - Per core, per segment: local GEMM1 (x @ w1w3) -> SwiGLU -> GEMM2 (h @ w2).
    All matmuls in fp32r (full PE rate, ~1e-4 rel err).
  - Host-side combine: scatter per-core output rows back to full output.

Layout choices:
  - x is pre-transposed on host to [HIDDEN, CAP] per core (hidden on
    partitions, tokens on the moving free dim).
  - w1w3 columns are permuted on host so that 128-row psum chunk c holds
    gate[64c:64c+64] on partitions 0:64 and up[64c:64c+64] on partitions
    64:128 -> SwiGLU is a partition-slice op, and h chunks line up for GEMM2.
  - GEMM2 uses h as the stationary operand ([inter, token] slices) and w2 as
    the moving operand -> output lands token-major [tok, HIDDEN] in PSUM and
    stores contiguously.
"""

import numpy as np

import concourse.bacc as bacc
import concourse.mybir as mybir
from concourse import tile
from concourse.bass_utils import run_bass_kernel_spmd

HIDDEN = 1024
INTER = 704
N_EXPERTS = 32
TOTAL_TOKENS = 8192
NCORES = 8
KC = HIDDEN // 128  # 8 k-chunks over hidden
MC = (2 * INTER) // 128  # 11 m-chunks over permuted gate|up dim
JC = (INTER + 127) // 128  # 6 k-chunks over inter for GEMM2 (last is 64 rows)
TT = 512  # token tile (moving free dim)

f32 = mybir.dt.float32
f32r = mybir.dt.float32r

# Column permutation of w1w3's last dim (2*INTER): chunk c of 128 holds
# gate[64c:64c+64] then up[64c:64c+64].
_PERM = np.empty(2 * INTER, dtype=np.int64)
for _c in range(MC):
    _PERM[128 * _c : 128 * _c + 64] = np.arange(64 * _c, 64 * _c + 64)
    _PERM[128 * _c + 64 : 128 * _c + 128] = INTER + np.arange(64 * _c, 64 * _c + 64)


def _plan(counts):
    """Balance (expert, token-chunk) pieces across NCORES cores.

    Returns (assign, caps, offs, cap_total):
      assign: per-core list of (expert, tok_start, n) with len == S (padded
              with (None, 0, 0) entries),
      caps:   per-slot capacity C_s (same across cores, multiples of 16),
      offs:   per-slot column offset into the capacity-CAP token buffer.
    """
    total = int(counts.sum())
    starts = np.zeros(N_EXPERTS, dtype=np.int64)
    np.cumsum(counts[:-1], out=starts[1:])

    target = max(1, -(-total // NCORES))
    chunks = []  # (n, expert, tok_start)
    for e in range(N_EXPERTS):
        n = int(counts[e])
        a = int(starts[e])
        if n <= 0:
            continue
        nparts = -(-n // target)
        base, rem = divmod(n, nparts)
        off = 0
        for p in range(nparts):
            ln = base + (1 if p < rem else 0)
            if ln > 0:
                chunks.append((ln, e, a + off))
                off += ln

    nchunks = max(1, len(chunks))
    S = -(-nchunks // NCORES)
    chunks.sort(reverse=True)

    per_core = [[] for _ in range(NCORES)]
    loads = [0] * NCORES
    for ch in chunks:
        cands = [c for c in range(NCORES) if len(per_core[c]) < S]
        c = min(cands, key=lambda i: loads[i])
        per_core[c].append(ch)
        loads[c] += ch[0]

    for c in range(NCORES):
        per_core[c].sort(reverse=True)
        while len(per_core[c]) < S:
            per_core[c].append((0, None, 0))

    caps = []
    for s in range(S):
        mx = max(per_core[c][s][0] for c in range(NCORES))
        caps.append(max(16, ((mx + 15) // 16) * 16))
    offs = np.concatenate([[0], np.cumsum(caps)[:-1]]).astype(np.int64)
    cap_total = int(sum(caps))

    assign = [
        [(e, a, n) for (n, e, a) in per_core[c]] for c in range(NCORES)
    ]
    return assign, caps, offs, cap_total


def _build(S, caps, offs, cap_total):
    """Build the SPMD Bass program for one core's segment structure."""
    nc = bacc.Bacc("TRN2", target_bir_lowering=False, debug=False, num_devices=NCORES)

    xt_d = nc.declare_dram_parameter("xt", [HIDDEN, cap_total], f32r, isOutput=False)
    w13_d = nc.declare_dram_parameter(
        "w13", [S, HIDDEN, 2 * INTER], f32r, isOutput=False
    )
    w2_d = nc.declare_dram_parameter("w2", [S, INTER, HIDDEN], f32r, isOutput=False)
    out_d = nc.declare_dram_parameter("out", [cap_total, HIDDEN], f32, isOutput=True)

    with tile.TileContext(nc) as tc:
        with (
            tc.tile_pool(name="w13p", bufs=12) as w13p,
            tc.tile_pool(name="w2p", bufs=8) as w2p,
            tc.tile_pool(name="xtp", bufs=10) as xtp,
            tc.tile_pool(name="hp", bufs=12) as hp,
            tc.tile_pool(name="sgp", bufs=4) as sgp,
            tc.tile_pool(name="outp", bufs=4) as outp,
            tc.tile_pool(name="ps1", bufs=4, space="PSUM") as ps1,
            tc.tile_pool(name="ps2", bufs=2, space="PSUM") as ps2,
        ):
            for s in range(S):
                C = caps[s]
                off = int(offs[s])

                w13_t = []
                for k in range(KC):
                    w13t = w13p.tile([128, 2 * INTER], f32r, tag="w13t")
                    nc.sync.dma_start(
                        out=w13t[:], in_=w13_d[s, 128 * k : 128 * (k + 1), :]
                    )
                    w13_t.append(w13t)
                w2_t = []
                for j in range(JC):
                    jw = min(128, INTER - 128 * j)
                    w2t = w2p.tile([jw, HIDDEN], f32r, tag="w2t", padded_shape=[128, HIDDEN])
                    nc.sync.dma_start(
                        out=w2t[:], in_=w2_d[s, 128 * j : 128 * j + jw, :]
                    )
                    w2_t.append(w2t)

                for t0 in range(0, C, TT):
                    tt = min(TT, C - t0)
                    xt_t = []
                    for k in range(KC):
                        xtt = xtp.tile([128, tt], f32r, tag="xtt", padded_shape=[128, TT])
                        nc.sync.dma_start(
                            out=xtt[:],
                            in_=xt_d[128 * k : 128 * (k + 1), off + t0 : off + t0 + tt],
                        )
                        xt_t.append(xtt)

                    h_t = []
                    for j in range(JC):
                        jw = min(128, INTER - 128 * j)
                        ht = hp.tile([jw, tt], f32r, tag="ht", padded_shape=[128, TT])
                        h_t.append(ht)

                    for m in range(MC):
                        pg = ps1.tile([128, tt], f32, tag="pg", padded_shape=[128, TT])
                        for k in range(KC):
                            nc.tensor.matmul(
                                pg[:],
                                w13_t[k][:, 128 * m : 128 * (m + 1)],
                                xt_t[k][:],
                                start=(k == 0),
                                stop=(k == KC - 1),
                            )
                        sg = sgp.tile([64, tt], f32, tag="sg", padded_shape=[64, TT])
                        nc.scalar.activation(
                            sg[:], pg[0:64, :], mybir.ActivationFunctionType.Silu
                        )
                        j, half = divmod(m, 2)
                        nc.vector.tensor_mul(
                            h_t[j][64 * half : 64 * half + 64, :], sg[:], pg[64:128, :]
                        )

                    for tc0 in range(0, tt, 128):
                        tw = min(128, tt - tc0)
                        po = ps2.tile([tw, HIDDEN], f32, tag="po", padded_shape=[128, HIDDEN])
                        for j in range(JC):
                            for nn in range(HIDDEN // 512):
                                nc.tensor.matmul(
                                    po[:, 512 * nn : 512 * (nn + 1)],
                                    h_t[j][:, tc0 : tc0 + tw],
                                    w2_t[j][:, 512 * nn : 512 * (nn + 1)],
                                    start=(j == 0),
                                    stop=(j == JC - 1),
                                )
                        ob = outp.tile([tw, HIDDEN], f32, tag="ob", padded_shape=[128, HIDDEN])
                        nc.vector.tensor_copy(ob[:], po[:])
                        nc.sync.dma_start(
                            out=out_d[off + t0 + tc0 : off + t0 + tc0 + tw, :],
                            in_=ob[:],
                        )

    nc.compile()
    return nc


_BUILD_CACHE = {}


def _get_program(S, caps, offs, cap_total):
    key = (S, tuple(caps))
    if key not in _BUILD_CACHE:
        _BUILD_CACHE[key] = _build(S, caps, offs, cap_total)
    return _BUILD_CACHE[key]


def _run(x, tokens_per_expert, w1w3, w2, trace=False):
    x = np.ascontiguousarray(np.asarray(x, dtype=np.float32))
    counts = np.asarray(tokens_per_expert, dtype=np.int64).copy()
    w1w3 = np.asarray(w1w3, dtype=np.float32)
    w2 = np.asarray(w2, dtype=np.float32)

    T = x.shape[0]
    # Clip group sizes like ragged_dot: groups are consecutive; anything
    # beyond T is out of range.
    counts = np.maximum(counts, 0)
    cum = np.cumsum(counts)
    over = cum > T
    if over.any():
        first = int(np.argmax(over))
        prev = int(cum[first - 1]) if first > 0 else 0
        counts[first] = T - prev
        counts[first + 1 :] = 0

    assign, caps, offs, cap_total = _plan(counts)
    S = len(caps)
    nc = _get_program(S, caps, offs, cap_total)

    w13_perm = w1w3[:, :, _PERM]  # [E, HIDDEN, 2*INTER]

    starts = np.zeros(N_EXPERTS, dtype=np.int64)
    np.cumsum(counts[:-1], out=starts[1:])

    in_maps = []
    for c in range(NCORES):
        xt_c = np.zeros((HIDDEN, cap_total), dtype=np.float32)
        w13_c = np.zeros((S, HIDDEN, 2 * INTER), dtype=np.float32)
        w2_c = np.zeros((S, INTER, HIDDEN), dtype=np.float32)
        for s, (e, a, n) in enumerate(assign[c]):
            if e is None or n <= 0:
                continue
            o = int(offs[s])
            xt_c[:, o : o + n] = x[a : a + n, :].T
            w13_c[s] = w13_perm[e]
            w2_c[s] = w2[e]
        in_maps.append({"xt": xt_c, "w13": w13_c, "w2": w2_c})

    res = run_bass_kernel_spmd(nc, in_maps, list(range(NCORES)), trace=trace)

    out_full = np.zeros((T, HIDDEN), dtype=np.float32)
    for c in range(NCORES):
        oc = res.results[c]["out"]
        for s, (e, a, n) in enumerate(assign[c]):
            if e is None or n <= 0:
                continue
            o = int(offs[s])
            out_full[a : a + n, :] = oc[o : o + n, :]
    return out_full, res


def kernel(x, tokens_per_expert, w1w3, w2, decoding=False, **_ignored):
    out, _ = _run(x, tokens_per_expert, w1w3, w2, trace=False)
    return out


# revision 3
# speedup vs baseline: 1.0092x; 1.0092x over previous
"""MoE grouped-GEMM expert FFN (SwiGLU) for Trainium2, 8-core expert parallelism.

Contract: kernel(**inputs) takes FULL unsharded inputs, returns FULL output.

Strategy:
  - Host-side routing: tokens are contiguous per expert; split expert groups
    into chunks, band-assign chunks across 8 cores with an identical
    segment-capacity structure on every core (SPMD: one Bass program).
  - Per core, per segment: local GEMM1 (x @ w1w3) -> SwiGLU -> GEMM2 (h @ w2).
    All matmuls in fp32r (full PE rate, ~1e-4 rel err vs fp32).
  - Host-side combine: scatter per-core output rows back to full output.

Layout choices:
  - x is pre-transposed on host to [HIDDEN, CAP] per core (hidden on
    partitions, tokens on the moving free dim).
  - w1w3 columns are permuted on host so that 128-row psum chunk c holds
    gate[64c:64c+64] on partitions 0:64 and up[64c:64c+64] on partitions
    64:128 -> SwiGLU is a partition-slice op, and h chunks line up for GEMM2.
  - GEMM1 iterates k (contraction chunk) outer / m inner within m-groups of
    <=4 so the first matmul only needs one 1MB DMA pair, and segment
    boundaries pipeline instead of stalling on the full 8.7MB weight load.
  - GEMM2 uses h as the stationary operand ([inter, token] slices) and w2 as
    the moving operand -> output lands token-major [tok, HIDDEN] in PSUM and
    stores contiguously.
"""

import numpy as np

import concourse.bacc as bacc
import concourse.mybir as mybir
from concourse import tile
from concourse.bass_utils import run_bass_kernel_spmd

HIDDEN = 1024
INTER = 704
N_EXPERTS = 32
NCORES = 8
KC = HIDDEN // 128  # 8 k-chunks over hidden
MC = (2 * INTER) // 128  # 11 m-chunks over permuted gate|up dim
JC = (INTER + 127) // 128  # 6 k-chunks over inter for GEMM2 (last is 64 rows)
TT = 512  # token tile (moving free dim)
M_GROUPS = [(0, 4), (4, 8), (8, 11)]  # m-ranges; <=4 psum banks live at once

f32 = mybir.dt.float32
f32r = mybir.dt.float32r

W13_BYTES = HIDDEN * 2 * INTER * 4
W2_BYTES = INTER * HIDDEN * 4

# Column permutation of w1w3's last dim (2*INTER): chunk c of 128 holds
# gate[64c:64c+64] then up[64c:64c+64].
_PERM = np.empty(2 * INTER, dtype=np.int64)
for _c in range(MC):
    _PERM[128 * _c : 128 * _c + 64] = np.arange(64 * _c, 64 * _c + 64)
    _PERM[128 * _c + 64 : 128 * _c + 128] = INTER + np.arange(64 * _c, 64 * _c + 64)


def _make_chunks(counts, starts, tmax):
    chunks = []  # (n, expert, tok_start)
    for e in range(N_EXPERTS):
        n = int(counts[e])
        a = int(starts[e])
        if n <= 0:
            continue
        nparts = -(-n // tmax)
        base, rem = divmod(n, nparts)
        off = 0
        for p in range(nparts):
            ln = base + (1 if p < rem else 0)
            if ln > 0:
                chunks.append((ln, e, a + off))
                off += ln
    return chunks


def _plan(counts):
    """Balance (expert, token-chunk) pieces across NCORES cores.

    Chunks are sorted by size and dealt in bands of 8 (one per core): slot s
    capacity = the largest chunk in band s, which minimizes total capacity
    for a given chunk multiset. The split threshold trades segment count
    (weight DMA traffic) against padding (PE + activation traffic).
    """
    total = int(counts.sum())
    starts = np.zeros(N_EXPERTS, dtype=np.int64)
    np.cumsum(counts[:-1], out=starts[1:])

    best = None
    for tmax in (4096, 2048, 1024, 768, 640, 512, 448, 384):
        chunks = _make_chunks(counts, starts, max(1, tmax))
        if not chunks:
            chunks = [(0, None, 0)]
        chunks.sort(key=lambda c: -c[0])
        S = -(-len(chunks) // NCORES)
        caps = []
        for s in range(S):
            band = chunks[NCORES * s : NCORES * (s + 1)]
            caps.append(max(16, ((band[0][0] + 15) // 16) * 16))
        cap_total = sum(caps)
        dma_t = (S * (W13_BYTES + W2_BYTES) + cap_total * 2 * HIDDEN * 4) / 331e9
        n_tiles = sum(-(-c // TT) for c in caps)
        pe_t = cap_total * 0.266 * 213e-9 + n_tiles * 88 * 60e-9
        score = max(dma_t, pe_t) + 0.2 * min(dma_t, pe_t)
        if best is None or score < best[0]:
            best = (score, chunks, S, caps)

    _, chunks, S, caps = best
    offs = np.concatenate([[0], np.cumsum(caps)[:-1]]).astype(np.int64)
    cap_total = int(sum(caps))

    assign = [[] for _ in range(NCORES)]
    for s in range(S):
        band = chunks[NCORES * s : NCORES * (s + 1)]
        for c in range(NCORES):
            if c < len(band):
                n, e, a = band[c]
                assign[c].append((e, a, n))
            else:
                assign[c].append((None, 0, 0))
    return assign, caps, offs, cap_total


def _build(S, caps, offs, cap_total):
    """Build the SPMD Bass program for one core's segment structure."""
    nc = bacc.Bacc("TRN2", target_bir_lowering=False, debug=False, num_devices=NCORES)

    xt_d = nc.declare_dram_parameter("xt", [HIDDEN, cap_total], f32r, isOutput=False)
    w13_d = nc.declare_dram_parameter(
        "w13", [S, HIDDEN, 2 * INTER], f32r, isOutput=False
    )
    w2_d = nc.declare_dram_parameter("w2", [S, INTER, HIDDEN], f32r, isOutput=False)
    out_d = nc.declare_dram_parameter("out", [cap_total, HIDDEN], f32, isOutput=True)

    with tile.TileContext(nc) as tc:
        with (
            tc.tile_pool(name="w13p", bufs=10) as w13p,
            tc.tile_pool(name="w2p", bufs=9) as w2p,
            tc.tile_pool(name="xtp", bufs=16) as xtp,
            tc.tile_pool(name="hp", bufs=8) as hp,
            tc.tile_pool(name="sgp", bufs=4) as sgp,
            tc.tile_pool(name="outp", bufs=4) as outp,
            tc.tile_pool(name="ps1", bufs=4, space="PSUM") as ps1,
            tc.tile_pool(name="ps2", bufs=2, space="PSUM") as ps2,
        ):
            for s in range(S):
                C = caps[s]
                off = int(offs[s])
                n_tt = -(-C // TT)

                # Weights for this segment (prefetched across segment
                # boundaries by pool buffering; k-order matches first use).
                w13_t = []
                for k in range(KC):
                    w13t = w13p.tile([128, 2 * INTER], f32r, tag="w13t")
                    nc.sync.dma_start(
                        out=w13t[:], in_=w13_d[s, 128 * k : 128 * (k + 1), :]
                    )
                    w13_t.append(w13t)
                w2_t = []
                for j in range(JC):
                    jw = min(128, INTER - 128 * j)
                    w2t = w2p.tile([jw, HIDDEN], f32r, tag="w2t",
                                   padded_shape=[128, HIDDEN])
                    nc.sync.dma_start(
                        out=w2t[:], in_=w2_d[s, 128 * j : 128 * j + jw, :]
                    )
                    w2_t.append(w2t)

                for t0 in range(0, C, TT):
                    tt = min(TT, C - t0)
                    xt_t = []
                    for k in range(KC):
                        xtt = xtp.tile([128, tt], f32r, tag="xtt",
                                       padded_shape=[128, TT])
                        nc.sync.dma_start(
                            out=xtt[:],
                            in_=xt_d[
                                128 * k : 128 * (k + 1), off + t0 : off + t0 + tt
                            ],
                        )
                        xt_t.append(xtt)

                    h_t = []
                    for j in range(JC):
                        jw = min(128, INTER - 128 * j)
                        ht = hp.tile([jw, tt], f32r, tag="ht",
                                     padded_shape=[128, TT])
                        h_t.append(ht)

                    # GEMM1: k-outer within m-groups of <=4 so PE work starts
                    # after the first (w13 chunk, xt chunk) pair lands.
                    for m_lo, m_hi in M_GROUPS:
                        pgs = {}
                        for m in range(m_lo, m_hi):
                            pgs[m] = ps1.tile([128, tt], f32, tag="pg",
                                              name=f"pg{m}",
                                              padded_shape=[128, TT])
                        for k in range(KC):
                            for m in range(m_lo, m_hi):
                                nc.tensor.matmul(
                                    pgs[m][:],
                                    w13_t[k][:, 128 * m : 128 * (m + 1)],
                                    xt_t[k][:],
                                    start=(k == 0),
                                    stop=(k == KC - 1),
                                )
                        for m in range(m_lo, m_hi):
                            sg = sgp.tile([64, tt], f32, tag="sg",
                                          padded_shape=[64, TT])
                            nc.scalar.activation(
                                sg[:], pgs[m][0:64, :],
                                mybir.ActivationFunctionType.Silu,
                            )
                            j, half = divmod(m, 2)
                            nc.vector.tensor_mul(
                                h_t[j][64 * half : 64 * half + 64, :],
                                sg[:],
                                pgs[m][64:128, :],
                            )

                    # GEMM2: h stationary, w2 moving; token-major output.
                    for tc0 in range(0, tt, 128):
                        tw = min(128, tt - tc0)
                        po = ps2.tile([tw, HIDDEN], f32, tag="po",
                                      padded_shape=[128, HIDDEN])
                        for j in range(JC):
                            for nn in range(HIDDEN // 512):
                                nc.tensor.matmul(
                                    po[:, 512 * nn : 512 * (nn + 1)],
                                    h_t[j][:, tc0 : tc0 + tw],
                                    w2_t[j][:, 512 * nn : 512 * (nn + 1)],
                                    start=(j == 0),
                                    stop=(j == JC - 1),
                                )
                        ob = outp.tile([tw, HIDDEN], f32, tag="ob",
                                       padded_shape=[128, HIDDEN])
                        nc.vector.tensor_copy(ob[:], po[:])
                        nc.sync.dma_start(
                            out=out_d[off + t0 + tc0 : off + t0 + tc0 + tw, :],
                            in_=ob[:],
                        )

    nc.compile()
    return nc


_BUILD_CACHE = {}


def _get_program(S, caps, offs, cap_total):
    key = (S, tuple(caps))
    if key not in _BUILD_CACHE:
        _BUILD_CACHE[key] = _build(S, caps, offs, cap_total)
    return _BUILD_CACHE[key]


def _run(x, tokens_per_expert, w1w3, w2, trace=False):
    x = np.ascontiguousarray(np.asarray(x, dtype=np.float32))
    counts = np.asarray(tokens_per_expert, dtype=np.int64).copy()
    w1w3 = np.asarray(w1w3, dtype=np.float32)
    w2 = np.asarray(w2, dtype=np.float32)

    T = x.shape[0]
    # Clip group sizes like ragged_dot: groups are consecutive; anything
    # beyond T is out of range.
    counts = np.maximum(counts, 0)
    cum = np.cumsum(counts)
    over = cum > T
    if over.any():
        first = int(np.argmax(over))
        prev = int(cum[first - 1]) if first > 0 else 0
        counts[first] = T - prev
        counts[first + 1 :] = 0

    assign, caps, offs, cap_total = _plan(counts)
    S = len(caps)
    nc = _get_program(S, caps, offs, cap_total)

    w13_perm = w1w3[:, :, _PERM]  # [E, HIDDEN, 2*INTER]

    in_maps = []
    for c in range(NCORES):
        xt_c = np.zeros((HIDDEN, cap_total), dtype=np.float32)
        w13_c = np.zeros((S, HIDDEN, 2 * INTER), dtype=np.float32)
        w2_c = np.zeros((S, INTER, HIDDEN), dtype=np.float32)
        for s, (e, a, n) in enumerate(assign[c]):
            if e is None or n <= 0:
                continue
            o = int(offs[s])
            xt_c[:, o : o + n] = x[a : a + n, :].T
            w13_c[s] = w13_perm[e]
            w2_c[s] = w2[e]
        in_maps.append({"xt": xt_c, "w13": w13_c, "w2": w2_c})

    res = run_bass_kernel_spmd(nc, in_maps, list(range(NCORES)), trace=trace)

    out_full = np.zeros((T, HIDDEN), dtype=np.float32)
    for c in range(NCORES):
        oc = res.results[c]["out"]
        for s, (e, a, n) in enumerate(assign[c]):
            if e is None or n <= 0:
                continue
            o = int(offs[s])
            out_full[a : a + n, :] = oc[o : o + n, :]
    return out_full, res


def kernel(x, tokens_per_expert, w1w3, w2, decoding=False, **_ignored):
    out, _ = _run(x, tokens_per_expert, w1w3, w2, trace=False)
    return out
